# revision 1
# baseline (speedup 1.0000x reference)
"""AdaProp GNN message-passing kernel for 8 TRN2 NeuronCores.

Strategy (v2 — collective-free): edges are sharded by destination-node range
(6250 nodes per core) so the segment-sum is fully local. Every core receives
the FULL transposed hidden state and builds the full projection table
  hG   = [hidden @ Ws' | hidden @ Wh]   [50176, 256] bf16  (A/B split halves)
locally on the TensorEngine (no AllGather). |Wa| is folded into Ws/Wr/Wqr/b
columns, which are permuted so positive-sign Wa columns come first; the
attention logit is then two 4x-mode tensor_scalar relu-accumulates
(l1 - l2 = Wa . relu(pre)). The relation table
  crel = [rela@Wr' + hqr' (by rel*64+ridx) | rela@Wh]   [25728, 256] bf16
is built by PE matmuls against constant selector matrices. Per edge, two
512-byte dma_gather rows (hG by sub, crel by rel*64+r_idx) are fetched and
summed in place on DVE; the alpha-scaled one-hot of the destination node is a
single two-scalar tensor_scalar (is_equal, mult); the segment sum is one
PSUM-accumulated matmul per tile; relu rides the Activation-engine eviction.
"""

import numpy as np

N, E, B, D = 50000, 500_000, 64, 128
NCORES = 8
NPC = 6250              # output nodes per core
WIN = 128               # nodes per PSUM window
NWIN = (NPC + WIN - 1) // WIN           # 49 windows per core
OUT_ROWS = NWIN * WIN                   # 6272 output rows per core
ROWS_T = 50176                          # hG table rows (50000 padded)
HALF = ROWS_T // 2                      # 25088 (< 32768 so int16 idx works)
NT_H = HALF // 128                      # 196 tiles per half table
BCH = 14                                # hG build tiles per DMA batch (196=14*14)
CREL_T = 201                            # crel tiles (201*128 = 25728 >= 401*64)
CREL_ROWS = CREL_T * 128
G = 3                                   # windows per gather group
MAXI = 1024                             # max idxs per dma_gather call (HW ucode limit)
P = 128


def _host_shard(edges):
    sub = np.asarray(edges[:, 4], dtype=np.int64)
    rel = np.asarray(edges[:, 2], dtype=np.int64)
    obj = np.asarray(edges[:, 5], dtype=np.int64)
    ridx = np.asarray(edges[:, 0], dtype=np.int64)

    core = obj // NPC
    loc = obj - core * NPC
    win = loc // WIN
    sel = loc - win * WIN
    half = (sub >= HALF).astype(np.int64)

    # per (core, window, half) edge index lists
    lists = [[[None, None] for _ in range(NWIN)] for _ in range(NCORES)]
    for k in range(NCORES):
        mk = np.nonzero(core == k)[0]
        key = win[mk] * 2 + half[mk]
        order = np.argsort(key, kind="stable")
        mk = mk[order]
        key = key[order]
        bounds = np.searchsorted(key, np.arange(2 * NWIN + 1))
        for w in range(NWIN):
            lists[k][w][0] = mk[bounds[2 * w]:bounds[2 * w + 1]]
            lists[k][w][1] = mk[bounds[2 * w + 1]:bounds[2 * w + 2]]

    # global per-(window,half) tile counts -> identical SPMD graph on all cores
    tcA = [max(len(lists[k][w][0]) for k in range(NCORES)) for w in range(NWIN)]
    tcB = [max(len(lists[k][w][1]) for k in range(NCORES)) for w in range(NWIN)]
    tcA = [(n + P - 1) // P for n in tcA]
    tcB = [(n + P - 1) // P for n in tcB]
    for w in range(NWIN):
        if tcA[w] + tcB[w] == 0:
            tcA[w] = 1

    # groups of G windows; tile stream per group: [A tiles][B tiles]
    groups = []          # (c_start, tA, tB)
    tile_window = []
    c = 0
    for g0 in range(0, NWIN, G):
        ws = list(range(g0, min(g0 + G, NWIN)))
        tA = sum(tcA[w] for w in ws)
        tB = sum(tcB[w] for w in ws)
        for w in ws:
            tile_window += [w] * tcA[w]
        for w in ws:
            tile_window += [w] * tcB[w]
        groups.append((c, tA, tB))
        c += tA + tB
    ctot = c
    S = ctot * P // 16   # idx array columns

    subs16 = np.zeros((NCORES, 16, S), dtype=np.int16)
    rels16 = np.zeros((NCORES, 16, S), dtype=np.int16)
    objs = np.full((NCORES, P, ctot), -1.0, dtype=np.float32)

    for k in range(NCORES):
        gi = 0
        for g0 in range(0, NWIN, G):
            ws = list(range(g0, min(g0 + G, NWIN)))
            c_start, tA, tB = groups[gi]
            gi += 1
            s0 = c_start * P // 16        # idx column base of this group
            n_all = (tA + tB) * P
            nA = tA * P

            # build the group's slot-ordered edge list (A runs then B runs)
            slot_sub = np.zeros(n_all, dtype=np.int64)
            slot_rel = np.zeros(n_all, dtype=np.int64)
            slot_obj = np.full(n_all, -1.0, dtype=np.float32)
            pos = 0
            for h, tc in ((0, tcA), (1, tcB)):
                for w in ws:
                    idx = lists[k][w][h]
                    n = len(idx)
                    nt = tc[w] * P
                    if n:
                        slot_sub[pos:pos + n] = sub[idx]
                        slot_rel[pos:pos + n] = rel[idx] * 64 + ridx[idx]
                        slot_obj[pos:pos + n] = sel[idx]
                    # pad slots: harmless gather target in the right half
                    slot_sub[pos + n:pos + nt] = 0 if h == 0 else HALF
                    pos += nt

            # per-slot arrays in [p, c] layout (slot j -> p=j%128, c=j//128)
            j = np.arange(n_all)
            objs[k, j % P, c_start + j // P] = slot_obj
            # idx arrays in 16-partition wrap, one wrap run per half segment
            jA = np.arange(nA)
            jB = np.arange(n_all - nA)
            jG = np.arange(n_all)
            subs16[k, jA % 16, s0 + jA // 16] = slot_sub[:nA]
            subs16[k, jB % 16, s0 + nA // 16 + jB // 16] = slot_sub[nA:] - HALF
            rels16[k, jG % 16, s0 + jG // 16] = slot_rel

    subs16 = np.tile(subs16, (1, 8, 1))   # replicate to 128 partitions
    rels16 = np.tile(rels16, (1, 8, 1))
    return subs16, rels16, objs, tile_window, groups, ctot


DEBUG_OUTPUTS = False


def _build_graph(ctot, tile_window, groups, kpos):
    import concourse.bass as bass
    import concourse.bacc as bacc
    import concourse.mybir as mybir
    from concourse.tile import TileContext

    f32 = mybir.dt.float32
    bf16 = mybir.dt.bfloat16
    i16 = mybir.dt.int16
    AF = mybir.ActivationFunctionType
    Alu = mybir.AluOpType

    S = ctot * P // 16
    assert 2 <= kpos <= 126

    nc = bacc.Bacc(dynamic_dma_scratch_size=65536)
    hidT = nc.declare_dram_parameter("hidT", [P, ROWS_T], bf16, isOutput=False)
    relaT = nc.declare_dram_parameter("relaT", [P, 512], bf16, isOutput=False)
    qrelT = nc.declare_dram_parameter("qrelT", [P, 64], bf16, isOutput=False)
    ws_p = nc.declare_dram_parameter("ws_p", [D, D], bf16, isOutput=False)
    wr_p = nc.declare_dram_parameter("wr_p", [D, D], bf16, isOutput=False)
    wh_p = nc.declare_dram_parameter("wh_p", [D, D], bf16, isOutput=False)
    wqr_p = nc.declare_dram_parameter("wqr_p", [D, D], bf16, isOutput=False)
    b_p = nc.declare_dram_parameter("b_p", [1, D], bf16, isOutput=False)
    sub_i = nc.declare_dram_parameter("sub_i", [P, S], i16, isOutput=False)
    rel_i = nc.declare_dram_parameter("rel_i", [P, S], i16, isOutput=False)
    obj_f = nc.declare_dram_parameter("obj_f", [P, ctot], f32, isOutput=False)
    out_ext = nc.declare_dram_parameter("out", [OUT_ROWS, D], f32, isOutput=True)

    first_tile = {}
    last_tile = {}
    for c, w in enumerate(tile_window):
        if w not in first_tile:
            first_tile[w] = c
        last_tile[w] = c

    with TileContext(nc) as tc:
        with (
            tc.tile_pool(name="const", bufs=1) as cpool,
            tc.tile_pool(name="dram", bufs=1, space="DRAM") as dpool,
            tc.tile_pool(name="work", bufs=2) as wpool,
            tc.tile_pool(name="psum", bufs=4, space="PSUM") as ppool,
            tc.tile_pool(name="aggp", bufs=4, space="PSUM") as apool,
        ):
            # ---- constants ----
            chan_i = cpool.tile([P, 1], mybir.dt.int32)
            nc.gpsimd.iota(chan_i[:], pattern=[[0, 1]], base=0, channel_multiplier=1)
            chan_f = cpool.tile([P, 1], f32)
            nc.vector.tensor_copy(chan_f[:], chan_i[:])

            iota_i = cpool.tile([P, P], i16)
            nc.gpsimd.iota(iota_i[:], pattern=[[1, P]], base=0, channel_multiplier=0)
            iota_b = cpool.tile([P, P], bf16)
            nc.vector.tensor_copy(iota_b[:], iota_i[:])

            # E64[k, 64k'+b] = (k == k') (crel rel selector, sliced on free dim)
            e64 = cpool.tile([P, P * 64], bf16)
            e_scr = cpool.tile([P, 16 * 64], i16)
            e_scrb = cpool.tile([P, 16 * 64], bf16)
            for q in range(8):
                nc.gpsimd.iota(e_scr[:], pattern=[[1, 16], [0, 64]],
                               base=16 * q, channel_multiplier=0)
                nc.vector.tensor_copy(e_scrb[:], e_scr[:])
                nc.vector.tensor_scalar(
                    out=e64[:, q * 1024:(q + 1) * 1024], in0=e_scrb[:],
                    scalar1=chan_f[:], scalar2=None, op0=Alu.is_equal)

            # I64dup[q, j] = (q == j % 64)
            i64_i = cpool.tile([64, P], i16)
            nc.gpsimd.iota(i64_i[:], pattern=[[0, 2], [1, 64]], base=0,
                           channel_multiplier=0)
            i64_b = cpool.tile([64, P], bf16)
            nc.vector.tensor_copy(i64_b[:], i64_i[:])
            i64dup = cpool.tile([64, P], bf16)
            nc.vector.tensor_scalar(out=i64dup[:], in0=i64_b[:],
                                    scalar1=chan_f[0:64, :], scalar2=None,
                                    op0=Alu.is_equal)

            ones64 = cpool.tile([1, 64], bf16)
            nc.gpsimd.memset(ones64[:], 1.0)

            # ---- weights ----
            wcat_g = cpool.tile([P, 2 * D], bf16)
            nc.sync.dma_start(out=wcat_g[:, 0:D], in_=ws_p[:])
            nc.sync.dma_start(out=wcat_g[:, D:2 * D], in_=wh_p[:])
            wcat_r = cpool.tile([P, 2 * D], bf16)
            nc.sync.dma_start(out=wcat_r[:, 0:D], in_=wr_p[:])
            nc.sync.dma_start(out=wcat_r[:, D:2 * D], in_=wh_p[:])
            wqr_sb = cpool.tile([P, D], bf16)
            nc.sync.dma_start(out=wqr_sb[:], in_=wqr_p[:])
            bp_sb = cpool.tile([1, D], bf16)
            nc.sync.dma_start(out=bp_sb[:], in_=b_p[:])
            qrelT_sb = cpool.tile([P, 64], bf16)
            nc.sync.dma_start(out=qrelT_sb[:], in_=qrelT[:])
            relaT_sb = cpool.tile([P, 512], bf16)
            nc.sync.dma_start(out=relaT_sb[:], in_=relaT[:])

            # ---- DRAM tables ----
            if DEBUG_OUTPUTS:
                hG_A = nc.declare_dram_parameter("dbg_hga", [HALF, 2 * D], bf16,
                                                 isOutput=True)
                hG_B = nc.declare_dram_parameter("dbg_hgb", [HALF, 2 * D], bf16,
                                                 isOutput=True)
                crel_d = nc.declare_dram_parameter("dbg_crel", [CREL_ROWS, 2 * D],
                                                   bf16, isOutput=True)
                dbg_l = nc.declare_dram_parameter("dbg_l", [P, 3 * ctot], f32,
                                                  isOutput=True)
            else:
                hG_A = dpool.tile([HALF, 2 * D], bf16)
                hG_B = dpool.tile([HALF, 2 * D], bf16)
                crel_d = dpool.tile([CREL_ROWS, 2 * D], bf16)

            # hrG in SBUF: [r_chunk, 256] x 4 chunks (512 rel rows padded)
            hrg_sb = cpool.tile([P, 4, 2 * D], bf16)
            for c in range(4):
                ps = ppool.tile([P, 2 * D], f32, tag="mm")
                nc.tensor.matmul(ps[:], lhsT=relaT_sb[:, c * P:(c + 1) * P],
                                 rhs=wcat_r[:], start=True, stop=True)
                nc.scalar.copy(hrg_sb[:, c, :], ps[:])

            # hqr' = qrel_sel @ Wqr' + b' -> [64, 256] (zero second half)
            hqr256 = cpool.tile([64, 2 * D], bf16)
            nc.gpsimd.memset(hqr256[:], 0.0)
            q_ps = ppool.tile([P, 2 * D], f32, tag="mm")
            nc.tensor.matmul(q_ps[0:64, 0:D], lhsT=qrelT_sb[:], rhs=wqr_sb[:],
                             start=True, stop=False)
            nc.tensor.matmul(q_ps[0:64, 0:D], lhsT=ones64[:], rhs=bp_sb[:],
                             start=False, stop=True)
            nc.scalar.copy(hqr256[:, 0:D], q_ps[0:64, 0:D])

            def batched_store(dst_tensor, row0, stage, nt):
                ap = bass.AP(dst_tensor, row0 * 2 * D,
                             [[2 * D, P], [P * 2 * D, nt], [1, 2 * D]])
                nc.sync.dma_start(out=ap, in_=stage[:, 0:nt, :])

            # ---- crel table build (PE); paired PSUM eviction on Act/DVE ----
            evict_n = 0

            def evict(dst_ap, src_ap):
                nonlocal evict_n
                if evict_n % 3 != 2:
                    nc.scalar.copy(dst_ap, src_ap)
                else:
                    nc.vector.tensor_copy(dst_ap, src_ap)
                evict_n += 1

            def emit_crel_chunk(t0):
                nt = min(8, CREL_T - t0)
                stage = wpool.tile([P, 8, 2 * D], bf16, tag="stage_c")
                for j in range(0, nt, 2):
                    np_ = min(2, nt - j)
                    ps = ppool.tile([P, np_ * 2 * D], f32, tag="mm")
                    for u in range(np_):
                        t = t0 + j + u
                        r0m = (2 * t) % P
                        chunk = (2 * t) // P
                        pcol = ps[:, u * 2 * D:(u + 1) * 2 * D]
                        nc.tensor.matmul(pcol, lhsT=e64[:, r0m * 64:r0m * 64 + P],
                                         rhs=hrg_sb[:, chunk, :],
                                         start=True, stop=False)
                        nc.tensor.matmul(pcol, lhsT=i64dup[:], rhs=hqr256[:],
                                         start=False, stop=True)
                    evict(stage[:, j:j + np_, :], ps[:])
                batched_store(crel_d[:].tensor, t0 * P, stage, nt)

            def emit_hg_chunk(half_i, b0):
                hG = hG_A if half_i == 0 else hG_B
                h_t = wpool.tile([P, BCH * P], bf16, tag="h_in")
                col0 = (half_i * NT_H + b0) * P
                nt = min(BCH, NT_H - b0)
                nc.sync.dma_start(out=h_t[:, 0:nt * P],
                                  in_=hidT[:, col0:col0 + nt * P])
                stage = wpool.tile([P, BCH, 2 * D], bf16, tag="stage_g")
                for j in range(0, nt, 2):
                    ps = ppool.tile([P, 2 * 2 * D], f32, tag="mm")
                    for u in range(2):
                        nc.tensor.matmul(
                            ps[:, u * 2 * D:(u + 1) * 2 * D],
                            lhsT=h_t[:, (j + u) * P:(j + u + 1) * P],
                            rhs=wcat_g[:], start=True, stop=True)
                    evict(stage[:, j:j + 2, :], ps[:])
                batched_store(hG[:].tensor, b0 * P, stage, nt)

            crel_chunks = list(range(0, CREL_T, 8))
            a_chunks = [(0, b0) for b0 in range(0, NT_H, BCH)]
            ci, gi2 = 0, 0
            while ci < len(crel_chunks) or gi2 < len(a_chunks):
                if gi2 < len(a_chunks):
                    emit_hg_chunk(*a_chunks[gi2]); gi2 += 1
                if ci < len(crel_chunks):
                    emit_crel_chunk(crel_chunks[ci]); ci += 1
                if ci < len(crel_chunks):
                    emit_crel_chunk(crel_chunks[ci]); ci += 1
            for b0 in range(0, NT_H, BCH):
                emit_hg_chunk(1, b0)

            # ---- edge index arrays ----
            sub_s = cpool.tile([P, S], i16)
            nc.sync.dma_start(out=sub_s[:], in_=sub_i[:])
            rel_s = cpool.tile([P, S], i16)
            nc.sync.dma_start(out=rel_s[:], in_=rel_i[:])
            obj_s = cpool.tile([P, ctot], f32)
            nc.sync.dma_start(out=obj_s[:], in_=obj_f[:])

            # ---- per-edge-slot accumulators ----
            l1 = cpool.tile([P, ctot], f32)   # later: logit (l1 - l2) in place
            l2 = cpool.tile([P, ctot], f32)   # later: alpha in place

            def chunked_gather(dst_tile, src_ap, idxs_tile, idx_col0, t_off, n):
                done = 0
                while done < n:
                    cn = min(MAXI, n - done)
                    ct0 = t_off + done // P
                    nc.gpsimd.dma_gather(
                        out_ap=dst_tile[:, ct0:ct0 + cn // P, :],
                        in_ap=src_ap,
                        idxs_ap=idxs_tile[:, idx_col0 + done // 16:
                                          idx_col0 + (done + cn) // 16],
                        num_idxs=cn, num_idxs_reg=cn, elem_size=2 * D)
                    done += cn

            # ---- edge processing ----
            agg = {}
            for g_idx, (c_start, tA, tB) in enumerate(groups):
                g0w = g_idx * G
                T = tA + tB
                nA = tA * P
                nB = tB * P
                s0 = c_start * P // 16

                r_t = wpool.tile([P, T, 2 * D], bf16, tag="g_r", bufs=2)
                # s = g + r in place, one half-table segment at a time
                g_t = wpool.tile([P, T, 2 * D], bf16, tag="g_g", bufs=2)
                CT = MAXI // P
                if tA:
                    chunked_gather(r_t, crel_d[:], rel_s, s0, 0, nA)
                    chunked_gather(g_t, hG_A[:], sub_s, s0, 0, nA)
                    for q0 in range(0, tA, CT):
                        q1 = min(tA, q0 + CT)
                        nc.vector.tensor_tensor(out=r_t[:, q0:q1, 0:D],
                                                in0=r_t[:, q0:q1, 0:D],
                                                in1=g_t[:, q0:q1, 0:D], op=Alu.add)
                if tB:
                    chunked_gather(r_t, crel_d[:], rel_s, s0 + nA // 16, tA, nB)
                    chunked_gather(g_t, hG_B[:], sub_s, s0 + nA // 16, tA, nB)
                    for q0 in range(tA, T, CT):
                        q1 = min(T, q0 + CT)
                        nc.vector.tensor_tensor(out=r_t[:, q0:q1, 0:D],
                                                in0=r_t[:, q0:q1, 0:D],
                                                in1=g_t[:, q0:q1, 0:D], op=Alu.add)

                dump = wpool.tile([P, P], bf16, tag="dump")
                for c in range(T):
                    ct = c_start + c
                    nc.vector.tensor_scalar(
                        out=dump[:, 0:kpos], in0=r_t[:, c, 0:kpos],
                        scalar1=0.0, scalar2=0.0, op0=Alu.max, op1=Alu.add,
                        accum_out=l1[:, ct:ct + 1])
                    nc.vector.tensor_scalar(
                        out=dump[:, kpos:D], in0=r_t[:, c, kpos:D],
                        scalar1=0.0, scalar2=0.0, op0=Alu.max, op1=Alu.add,
                        accum_out=l2[:, ct:ct + 1])

                nc.vector.tensor_tensor(
                    out=l1[:, c_start:c_start + T], in0=l1[:, c_start:c_start + T],
                    in1=l2[:, c_start:c_start + T], op=Alu.subtract)
                nc.scalar.activation(l2[:, c_start:c_start + T],
                                     l1[:, c_start:c_start + T], AF.Sigmoid)

                nw = len(set(tile_window[c_start:c_start + T]))
                ostage = wpool.tile([P, G, D], f32, tag="ostage")
                for c in range(T):
                    ct = c_start + c
                    oh = wpool.tile([P, P], bf16, tag="oh", bufs=8)
                    nc.vector.tensor_scalar(
                        out=oh[:], in0=iota_b[:],
                        scalar1=obj_s[:, ct:ct + 1], scalar2=l2[:, ct:ct + 1],
                        op0=Alu.is_equal, op1=Alu.mult)
                    w = tile_window[ct]
                    if ct == first_tile[w]:
                        agg[w] = apool.tile([P, D], f32, tag="agg", name=f"agg_{w}")
                    nc.tensor.matmul(agg[w][:], lhsT=oh[:],
                                     rhs=g_t[:, c, D:2 * D],
                                     start=(ct == first_tile[w]), stop=False)
                    nc.tensor.matmul(agg[w][:], lhsT=oh[:],
                                     rhs=r_t[:, c, D:2 * D],
                                     start=False, stop=(ct == last_tile[w]))
                    if ct == last_tile[w]:
                        nc.scalar.activation(ostage[:, w - g0w, :], agg[w][:],
                                             AF.Relu)
                        del agg[w]
                out_ap = bass.AP(out_ext[:].tensor, g0w * P * D,
                                 [[D, P], [P * D, nw], [1, D]])
                nc.sync.dma_start(out=out_ap, in_=ostage[:, 0:nw, :])

            if DEBUG_OUTPUTS:
                nc.sync.dma_start(out=dbg_l[:, 0:ctot], in_=l1[:])
                nc.sync.dma_start(out=dbg_l[:, ctot:2 * ctot], in_=l2[:])
                nc.sync.dma_start(out=dbg_l[:, 2 * ctot:3 * ctot], in_=l2[:])

    nc.compile()
    return nc


def _prep_weights(Wa, Ws, Wr, Wqr_w, Wqr_b):
    import ml_dtypes
    wa = np.asarray(Wa, dtype=np.float64)
    pos = np.nonzero(wa >= 0)[0]
    neg = np.nonzero(wa < 0)[0]
    order = np.concatenate([pos, neg])
    kpos = len(pos)
    scale = np.abs(wa)[order]

    def prep(w):
        w = np.asarray(w, dtype=np.float64)[:, order] * scale[None, :]
        return np.ascontiguousarray(w.astype(ml_dtypes.bfloat16))

    ws_p = prep(Ws)
    wr_p = prep(Wr)
    wqr_p = prep(Wqr_w)
    b_p = (np.asarray(Wqr_b, dtype=np.float64)[order] * scale).reshape(1, D)
    b_p = np.ascontiguousarray(b_p.astype(ml_dtypes.bfloat16))
    return ws_p, wr_p, wqr_p, b_p, kpos


def prepare(q_rel, hidden, edges, rela_embed, Ws, Wr, Wqr_w, Wqr_b, Wa, Wh,
            n_node=None):
    """Build the Bass graph and the 8 per-core input maps."""
    import ml_dtypes

    q_rel = np.asarray(q_rel)
    hidden = np.asarray(hidden, dtype=np.float32)
    edges = np.asarray(edges)
    rela_embed = np.asarray(rela_embed, dtype=np.float32)

    subs16, rels16, objs, tile_window, groups, ctot = _host_shard(edges)
    ws_p, wr_p, wqr_p, b_p, kpos = _prep_weights(Wa, Ws, Wr, Wqr_w, Wqr_b)
    nc = _build_graph(ctot, tile_window, groups, kpos)

    bf = ml_dtypes.bfloat16
    hidT = np.zeros((D, ROWS_T), dtype=bf)
    hidT[:, :N] = hidden.T.astype(bf)
    relaT = np.zeros((D, 512), dtype=bf)
    relaT[:, :rela_embed.shape[0]] = rela_embed.T.astype(bf)
    qrelT = np.ascontiguousarray(
        rela_embed[np.asarray(q_rel, dtype=np.int64)].T.astype(bf))
    wh_b = np.ascontiguousarray(np.asarray(Wh, dtype=np.float32).astype(bf))

    in_maps = []
    for k in range(NCORES):
        in_maps.append({
            "hidT": hidT,
            "relaT": relaT,
            "qrelT": qrelT,
            "ws_p": ws_p,
            "wr_p": wr_p,
            "wh_p": wh_b,
            "wqr_p": wqr_p,
            "b_p": b_p,
            "sub_i": subs16[k],
            "rel_i": rels16[k],
            "obj_f": objs[k],
        })
    return nc, in_maps


def kernel(q_rel, hidden, edges, rela_embed, Ws, Wr, Wqr_w, Wqr_b, Wa, Wh, n_node):
    from concourse.bass_utils import run_bass_kernel_spmd

    nc, in_maps = prepare(q_rel, hidden, edges, rela_embed, Ws, Wr, Wqr_w,
                          Wqr_b, Wa, Wh, n_node)
    res = run_bass_kernel_spmd(nc, in_maps, list(range(NCORES)))
    out = np.concatenate([res.results[k]["out"][:NPC] for k in range(NCORES)],
                         axis=0)
    return out.astype(np.float32)


if __name__ == "__main__":
    import reference

    inputs = reference.setup_inputs()
    inputs = {k: np.asarray(v) for k, v in inputs.items()}
    got = kernel(**inputs)
    exp = np.asarray(reference.reference(**inputs))
    err = np.abs(got - exp).max() / (np.abs(exp).max() + 1e-9)
    print("rel err:", err)



# revision 5
# speedup vs baseline: 2.0470x; 2.0470x over previous
"""AdaProp GNN message-passing kernel for 8 TRN2 NeuronCores.

v4 = v3 graph + balanced host scheduling:
- nodes are assigned to cores/windows by degree-balancing (LPT + snake deal),
  with the output unpermuted on the host, so per-(window,half) slot targets
  are uniform across cores at 1-slot granularity (no 128-padding per window);
- only gather sections (the [A-half | B-half] runs of each G-window group)
  are 128-aligned; tiles straddle windows, handled by per-tile incidence
  lists (one one-hot + matmul pair per (tile, window) incidence).

Per edge, two dma_gather rows: hs = hidden_rm[sub] (256B, A/B halves) and
crel[rel*64+r_idx] = [rela@Wr + rela[q_rel]@Wqr + b | rela_raw] (512B,
host-built). Attention runs in transposed [a, slot] space (PE identity-
transpose + Ws matmul + identity-add of crel_attn), relu on Activation,
logit = relu(preT).T @ Wa on PE (slot-partitioned), sigmoid per chunk.
Message aggregate aggT[d, node] += msg.T @ oh in PSUM; out = relu(aggT.T@Wh).
"""

import numpy as np

N, E, B, D = 50000, 500_000, 64, 128
NCORES = 8
WIN = 128
NWIN = 49                               # windows per core
OUT_ROWS = NWIN * WIN                   # 6272 output rows per core
NPC = OUT_ROWS                          # rows fetched per core (permuted)
ROWS_T = 50176                          # hidden_rm rows (50000 padded)
HALF = ROWS_T // 2                      # 25088 (< 32768 so int16 idx works)
CREL_T = 201
CREL_ROWS = CREL_T * 128
G = 3                                   # windows per group
CH = 3                                  # tiles per attention chunk (PSUM bank)
MAXI = 1024                             # max idxs per dma_gather call (HW limit)
P = 128


def _host_shard(edges):
    sub = np.asarray(edges[:, 4], dtype=np.int64)
    rel = np.asarray(edges[:, 2], dtype=np.int64)
    obj = np.asarray(edges[:, 5], dtype=np.int64)
    ridx = np.asarray(edges[:, 0], dtype=np.int64)
    half = (sub >= HALF).astype(np.int64)

    degA = np.bincount(obj[half == 0], minlength=N)
    degB = np.bincount(obj[half == 1], minlength=N)
    tot = degA + degB

    # ---- nodes -> cores: LPT greedy on total degree, cap OUT_ROWS ----
    order = np.argsort(-tot, kind="stable")
    core_load = np.zeros(NCORES, dtype=np.int64)
    core_cnt = np.zeros(NCORES, dtype=np.int64)
    ncore = np.zeros(N, dtype=np.int64)
    for n in order:
        k = int(np.argmin(np.where(core_cnt < OUT_ROWS, core_load, 1 << 60)))
        ncore[n] = k
        core_load[k] += tot[n]
        core_cnt[k] += 1

    # ---- per core: snake-deal nodes (by degree desc) into 49 windows ----
    nwin = np.zeros(N, dtype=np.int64)
    npos = np.zeros(N, dtype=np.int64)
    rowmap = np.full((NCORES, OUT_ROWS), -1, dtype=np.int64)
    for k in range(NCORES):
        mine = np.nonzero(ncore == k)[0]
        mine = mine[np.argsort(-tot[mine], kind="stable")]
        wfill = np.zeros(NWIN, dtype=np.int64)
        wi = 0
        direction = 1
        for n in mine:
            # snake over windows, skipping full ones
            tries = 0
            while wfill[wi] >= WIN:
                wi += direction
                if wi == NWIN or wi < 0:
                    direction = -direction
                    wi += direction
                tries += 1
                assert tries <= 2 * NWIN
            nwin[n] = wi
            npos[n] = wfill[wi]
            rowmap[k, wi * WIN + wfill[wi]] = n
            wfill[wi] += 1
            wi += direction
            if wi == NWIN or wi < 0:
                direction = -direction
                wi += direction

    # ---- per (core, window, half) edge counts -> uniform targets ----
    ek = ncore[obj]
    ew = nwin[obj]
    key = (ek * NWIN + ew) * 2 + half
    cnts = np.bincount(key, minlength=NCORES * NWIN * 2).reshape(
        NCORES, NWIN, 2)
    tgt = cnts.max(axis=0)              # [NWIN, 2]

    # ---- uniform slot layout ----
    # per group: [w0A w1A w2A padA | w0B w1B w2B padB], sections 128-aligned
    ngrp = (NWIN + G - 1) // G
    offs = np.zeros((NWIN, 2), dtype=np.int64)   # stretch start slot
    groups = []     # (c_start_tile, tA, tB, wlist)
    slot_win = []   # per-slot window id or -1 (uniform)
    cur = 0
    for g in range(ngrp):
        wlist = list(range(g * G, min((g + 1) * G, NWIN)))
        c_start = cur // P
        secs = []
        for h in (0, 1):
            sec0 = cur
            for w in wlist:
                offs[w, h] = cur
                cur += int(tgt[w, h])
            pad = (-(cur - sec0)) % P
            cur += pad
            secs.append((cur - sec0) // P)
        groups.append((c_start, secs[0], secs[1], wlist))
        wmap = np.full(cur - c_start * P, -1, dtype=np.int64)
        for h in (0, 1):
            for w in wlist:
                o = offs[w, h] - c_start * P
                wmap[o:o + tgt[w, h]] = w
        slot_win.append(wmap)
    ctot = cur // P
    slot_win = np.concatenate(slot_win)
    S = ctot * P // 16

    # ---- per-tile incidence lists (uniform across cores) ----
    tile_inc = []   # per tile: list of (w, col)
    inc_of = {}
    ncol = 0
    for ct in range(ctot):
        ws = [int(w) for w in
              np.unique(slot_win[ct * P:(ct + 1) * P]) if w >= 0]
        lst = []
        for w in ws:
            lst.append((w, ncol))
            inc_of.setdefault(w, []).append((ct, ncol))
            ncol += 1
        tile_inc.append(lst)
    first_inc = {w: v[0] for w, v in inc_of.items()}
    last_inc = {w: v[-1] for w, v in inc_of.items()}

    # ---- per-core slot data ----
    subs16 = np.zeros((NCORES, 16, S), dtype=np.int16)
    rels16 = np.zeros((NCORES, 16, S), dtype=np.int16)
    objc = np.full((NCORES, P, ncol), -1.0, dtype=np.float32)

    esel = npos[obj]
    erelc = rel * 64 + ridx
    eorder = np.lexsort((half, ew, ek))
    ksort, wsort, hsort = ek[eorder], ew[eorder], half[eorder]
    bkey = (ksort * NWIN + wsort) * 2 + hsort
    bounds = np.searchsorted(bkey, np.arange(NCORES * NWIN * 2 + 1))

    def wrap(dst, col0, vals):
        j = np.arange(len(vals))
        dst[j % 16, col0 + j // 16] = vals

    for k in range(NCORES):
        sub_slot = np.zeros(ctot * P, dtype=np.int64)
        rel_slot = np.zeros(ctot * P, dtype=np.int64)
        sel_slot = np.full(ctot * P, -1.0, dtype=np.float32)
        for w in range(NWIN):
            for h in (0, 1):
                bi = (k * NWIN + w) * 2 + h
                eidx = eorder[bounds[bi]:bounds[bi + 1]]
                n = len(eidx)
                o = offs[w, h]
                sub_slot[o:o + n] = sub[eidx] - h * HALF
                rel_slot[o:o + n] = erelc[eidx]
                sel_slot[o:o + n] = esel[eidx]
        # idx arrays: one 16-wrap run per gather section
        for (c_start, tA, tB, wlist) in groups:
            b0 = c_start * P
            wrap(subs16[k], b0 // 16, sub_slot[b0:b0 + tA * P])
            wrap(subs16[k], (b0 + tA * P) // 16,
                 sub_slot[b0 + tA * P:b0 + (tA + tB) * P])
            wrap(rels16[k], b0 // 16, rel_slot[b0:b0 + (tA + tB) * P])
        # incidence one-hot select columns
        for ct in range(ctot):
            sl = slice(ct * P, (ct + 1) * P)
            for (w, col) in tile_inc[ct]:
                objc[k, :, col] = np.where(slot_win[sl] == w,
                                           sel_slot[sl], -1.0)

    subs16 = np.tile(subs16, (1, 8, 1))
    rels16 = np.tile(rels16, (1, 8, 1))
    sched = dict(groups=groups, tile_inc=tile_inc, first_inc=first_inc,
                 last_inc=last_inc, ctot=ctot, ncol=ncol)
    return subs16, rels16, objc, rowmap, sched


def _build_graph(sched):
    import concourse.bass as bass
    import concourse.bacc as bacc
    import concourse.mybir as mybir
    from concourse.tile import TileContext

    f32 = mybir.dt.float32
    bf16 = mybir.dt.bfloat16
    i16 = mybir.dt.int16
    AF = mybir.ActivationFunctionType
    Alu = mybir.AluOpType

    ctot = sched["ctot"]
    ncol = sched["ncol"]
    groups = sched["groups"]
    tile_inc = sched["tile_inc"]
    first_inc = sched["first_inc"]
    last_inc = sched["last_inc"]
    S = ctot * P // 16

    nc = bacc.Bacc(dynamic_dma_scratch_size=65536)
    hid_rm = nc.declare_dram_parameter("hid_rm", [ROWS_T, D], bf16, isOutput=False)
    crel_d = nc.declare_dram_parameter("crel", [CREL_ROWS, 2 * D], bf16,
                                       isOutput=False)
    ws_p = nc.declare_dram_parameter("ws_p", [D, D], bf16, isOutput=False)
    wh_p = nc.declare_dram_parameter("wh_p", [D, D], bf16, isOutput=False)
    wa_p = nc.declare_dram_parameter("wa_p", [D, 1], bf16, isOutput=False)
    sub_i = nc.declare_dram_parameter("sub_i", [P, S], i16, isOutput=False)
    rel_i = nc.declare_dram_parameter("rel_i", [P, S], i16, isOutput=False)
    obj_f = nc.declare_dram_parameter("obj_f", [P, ncol], f32, isOutput=False)
    out_ext = nc.declare_dram_parameter("out", [OUT_ROWS, D], bf16, isOutput=True)

    with TileContext(nc) as tc:
        with (
            tc.tile_pool(name="const", bufs=1) as cpool,
            tc.tile_pool(name="work", bufs=2) as wpool,
            tc.tile_pool(name="psum", bufs=2, space="PSUM") as ppool,
        ):
            # ---- constants ----
            chan_i = cpool.tile([P, 1], mybir.dt.int32)
            nc.gpsimd.iota(chan_i[:], pattern=[[0, 1]], base=0, channel_multiplier=1)
            chan_f = cpool.tile([P, 1], f32)
            nc.vector.tensor_copy(chan_f[:], chan_i[:])

            iota_i = cpool.tile([P, P], i16)
            nc.gpsimd.iota(iota_i[:], pattern=[[1, P]], base=0, channel_multiplier=0)
            iota_b = cpool.tile([P, P], bf16)
            nc.vector.tensor_copy(iota_b[:], iota_i[:])

            i128 = cpool.tile([P, P], bf16)
            nc.vector.tensor_scalar(out=i128[:], in0=iota_b[:],
                                    scalar1=chan_f[:], scalar2=None,
                                    op0=Alu.is_equal)

            # ---- weights ----
            ws_sb = cpool.tile([P, D], bf16)
            nc.sync.dma_start(out=ws_sb[:], in_=ws_p[:])
            wh_sb = cpool.tile([P, D], bf16)
            nc.sync.dma_start(out=wh_sb[:], in_=wh_p[:])
            wa_sb = cpool.tile([P, 1], bf16)
            nc.sync.dma_start(out=wa_sb[:], in_=wa_p[:])

            # ---- edge index arrays ----
            sub_s = cpool.tile([P, S], i16)
            nc.sync.dma_start(out=sub_s[:], in_=sub_i[:])
            rel_s = cpool.tile([P, S], i16)
            nc.sync.dma_start(out=rel_s[:], in_=rel_i[:])
            obj_s = cpool.tile([P, ncol], f32)
            nc.sync.dma_start(out=obj_s[:], in_=obj_f[:])

            def chunked_gather(dst_tile, src_ap, idxs_tile, idx_col0, t_off, n,
                               esize):
                done = 0
                while done < n:
                    cn = min(MAXI, n - done)
                    ct0 = t_off + done // P
                    nc.gpsimd.dma_gather(
                        out_ap=dst_tile[:, ct0:ct0 + cn // P, :],
                        in_ap=src_ap,
                        idxs_ap=idxs_tile[:, idx_col0 + done // 16:
                                          idx_col0 + (done + cn) // 16],
                        num_idxs=cn, num_idxs_reg=cn, elem_size=esize)
                    done += cn

            # ---- edge processing ----
            evict_n = 0
            agg = {}
            for g_idx, (c_start, tA, tB, wlist) in enumerate(groups):
                g0w = wlist[0]
                nw = len(wlist)
                T = tA + tB
                nA = tA * P
                nB = tB * P
                s0 = c_start * P // 16

                g_t = wpool.tile([P, T, D], bf16, tag="g_hs", bufs=3)
                r_t = wpool.tile([P, T, 2 * D], bf16, tag="g_cr", bufs=3)
                chunked_gather(r_t, crel_d[:], rel_s, s0, 0, nA + nB, 2 * D)
                if tA:
                    chunked_gather(g_t, hid_rm[0:HALF, :], sub_s, s0, 0, nA, D)
                if tB:
                    chunked_gather(g_t, hid_rm[HALF:ROWS_T, :], sub_s,
                                   s0 + nA // 16, tA, nB, D)

                # ---- attention ----
                alpha = wpool.tile([P, T], f32, tag="alpha", bufs=2)
                for j0 in range(0, T, CH):
                    nb = min(CH, T - j0)
                    psT = ppool.tile([P, CH, D], bf16, tag="psT", bufs=2)
                    for j in range(nb):
                        nc.tensor.matmul(psT[:, j, :], lhsT=g_t[:, j0 + j, :],
                                         rhs=i128[:], is_transpose=True,
                                         start=True, stop=True)
                    hsT_sb = wpool.tile([P, CH, D], bf16, tag="hsT", bufs=2)
                    nc.vector.tensor_copy(hsT_sb[:, 0:nb, :], psT[:, 0:nb, :])
                    preL = ppool.tile([P, CH * D + CH], f32, tag="preL", bufs=2)
                    for j in range(nb):
                        nc.tensor.matmul(preL[:, j * D:(j + 1) * D],
                                         lhsT=ws_sb[:], rhs=hsT_sb[:, j, :],
                                         start=True, stop=False)
                        nc.tensor.matmul(preL[:, j * D:(j + 1) * D],
                                         lhsT=r_t[:, j0 + j, 0:D],
                                         rhs=i128[:], start=False, stop=True)
                    s4 = wpool.tile([P, CH, D], bf16, tag="s4", bufs=2)
                    nc.scalar.activation(s4[:, 0:nb, :], preL[:, 0:nb * D],
                                         AF.Relu)
                    for j in range(nb):
                        nc.tensor.matmul(
                            preL[:, CH * D + j:CH * D + j + 1],
                            lhsT=s4[:, j, :], rhs=wa_sb[:],
                            start=True, stop=True)
                    nc.scalar.activation(alpha[:, j0:j0 + nb],
                                         preL[:, CH * D:CH * D + nb],
                                         AF.Sigmoid)

                # ---- aggregation over (tile, window) incidences ----
                for c in range(T):
                    ct = c_start + c
                    for (w, col) in tile_inc[ct]:
                        oh = wpool.tile([P, P], bf16, tag="oh", bufs=8)
                        nc.vector.tensor_scalar(
                            out=oh[:], in0=iota_b[:],
                            scalar1=obj_s[:, col:col + 1],
                            scalar2=alpha[:, c:c + 1],
                            op0=Alu.is_equal, op1=Alu.mult)
                        if (ct, col) == first_inc[w]:
                            agg[w] = ppool.tile([P, D], f32, tag="agg",
                                                bufs=4, name=f"agg_{w}")
                        nc.tensor.matmul(agg[w][:], lhsT=g_t[:, c, :],
                                         rhs=oh[:],
                                         start=((ct, col) == first_inc[w]),
                                         stop=False)
                        nc.tensor.matmul(agg[w][:], lhsT=r_t[:, c, D:2 * D],
                                         rhs=oh[:], start=False,
                                         stop=((ct, col) == last_inc[w]))

                # ---- epilogue: out = relu(aggT.T @ Wh) ----
                agg_sb = wpool.tile([P, G, D], bf16, tag="aggsb", bufs=2)
                for wi, w in enumerate(wlist):
                    if evict_n % 2 == 0:
                        nc.vector.tensor_copy(agg_sb[:, wi, :], agg[w][:])
                    else:
                        nc.scalar.copy(agg_sb[:, wi, :], agg[w][:])
                    evict_n += 1
                    del agg[w]
                outp = ppool.tile([P, CH * D + CH], f32, tag="preL", bufs=2)
                for wi in range(nw):
                    nc.tensor.matmul(outp[:, wi * D:(wi + 1) * D],
                                     lhsT=agg_sb[:, wi, :],
                                     rhs=wh_sb[:], start=True, stop=True)
                ostage = wpool.tile([P, G, D], bf16, tag="ostage", bufs=2)
                nc.scalar.activation(ostage[:, 0:nw, :], outp[:, 0:nw * D],
                                     AF.Relu)
                out_ap = bass.AP(out_ext[:].tensor, g0w * P * D,
                                 [[D, P], [P * D, nw], [1, D]])
                nc.sync.dma_start(out=out_ap, in_=ostage[:, 0:nw, :])

    nc.compile()
    return nc


def prepare(q_rel, hidden, edges, rela_embed, Ws, Wr, Wqr_w, Wqr_b, Wa, Wh,
            n_node=None):
    """Build the Bass graph and the 8 per-core input maps."""
    import ml_dtypes

    bf = ml_dtypes.bfloat16
    q_rel = np.asarray(q_rel, dtype=np.int64)
    hidden = np.asarray(hidden, dtype=np.float32)
    edges = np.asarray(edges)
    rela = np.asarray(rela_embed, dtype=np.float32)

    subs16, rels16, objc, rowmap, sched = _host_shard(edges)
    nc = _build_graph(sched)

    hid_rm = np.zeros((ROWS_T, D), dtype=bf)
    hid_rm[:N] = hidden.astype(bf)

    attn_rel = rela @ np.asarray(Wr, dtype=np.float32)
    attn_q = (rela[q_rel] @ np.asarray(Wqr_w, dtype=np.float32)
              + np.asarray(Wqr_b, dtype=np.float32))
    crel = np.zeros((CREL_ROWS, 2 * D), dtype=np.float32)
    nrel = rela.shape[0]
    crel_attn = (attn_rel[:, None, :] + attn_q[None, :, :])
    crel[:nrel * 64, 0:D] = crel_attn.reshape(nrel * 64, D)
    crel[:nrel * 64, D:2 * D] = np.repeat(rela, 64, axis=0)
    crel = np.ascontiguousarray(crel.astype(bf))

    ws_b = np.ascontiguousarray(np.asarray(Ws, dtype=np.float32).astype(bf))
    wh_b = np.ascontiguousarray(np.asarray(Wh, dtype=np.float32).astype(bf))
    wa_b = np.ascontiguousarray(
        np.asarray(Wa, dtype=np.float32).reshape(D, 1).astype(bf))

    in_maps = []
    for k in range(NCORES):
        in_maps.append({
            "hid_rm": hid_rm,
            "crel": crel,
            "ws_p": ws_b,
            "wh_p": wh_b,
            "wa_p": wa_b,
            "sub_i": subs16[k],
            "rel_i": rels16[k],
            "obj_f": objc[k],
        })
    return nc, in_maps, rowmap


def assemble(results, rowmap):
    out = np.zeros((N, D), dtype=np.float32)
    for k in range(NCORES):
        rows = np.asarray(results[k]["out"], dtype=np.float32)
        valid = rowmap[k] >= 0
        out[rowmap[k][valid]] = rows[valid]
    return out


def kernel(q_rel, hidden, edges, rela_embed, Ws, Wr, Wqr_w, Wqr_b, Wa, Wh, n_node):
    from concourse.bass_utils import run_bass_kernel_spmd

    nc, in_maps, rowmap = prepare(q_rel, hidden, edges, rela_embed, Ws, Wr,
                                  Wqr_w, Wqr_b, Wa, Wh, n_node)
    res = run_bass_kernel_spmd(nc, in_maps, list(range(NCORES)))
    return assemble(res.results, rowmap)


if __name__ == "__main__":
    import reference

    inputs = reference.setup_inputs()
    inputs = {k: np.asarray(v) for k, v in inputs.items()}
    got = kernel(**inputs)
    exp = np.asarray(reference.reference(**inputs))
    err = np.abs(got - exp).max() / (np.abs(exp).max() + 1e-9)
    print("rel err:", err)


# revision 6
# speedup vs baseline: 2.1073x; 1.0294x over previous
"""AdaProp GNN message-passing kernel for 8 TRN2 NeuronCores.

v4 = v3 graph + balanced host scheduling:
- nodes are assigned to cores/windows by degree-balancing (LPT + snake deal),
  with the output unpermuted on the host, so per-(window,half) slot targets
  are uniform across cores at 1-slot granularity (no 128-padding per window);
- only gather sections (the [A-half | B-half] runs of each G-window group)
  are 128-aligned; tiles straddle windows, handled by per-tile incidence
  lists (one one-hot + matmul pair per (tile, window) incidence).

Per edge, two dma_gather rows: hs = hidden_rm[sub] (256B, A/B halves) and
crel[rel*64+r_idx] = [rela@Wr + rela[q_rel]@Wqr + b | rela_raw] (512B,
host-built). Attention runs in transposed [a, slot] space (PE identity-
transpose + Ws matmul + identity-add of crel_attn), relu on Activation,
logit = relu(preT).T @ Wa on PE (slot-partitioned), sigmoid per chunk.
Message aggregate aggT[d, node] += msg.T @ oh in PSUM; out = relu(aggT.T@Wh).
"""

import numpy as np

N, E, B, D = 50000, 500_000, 64, 128
NCORES = 8
WIN = 128
NWIN = 49                               # windows per core
OUT_ROWS = NWIN * WIN                   # 6272 output rows per core
NPC = OUT_ROWS                          # rows fetched per core (permuted)
ROWS_T = 50176                          # hidden_rm rows (50000 padded)
HALF = ROWS_T // 2                      # 25088 (< 32768 so int16 idx works)
CREL_T = 201
CREL_ROWS = CREL_T * 128
G = 3                                   # windows per group
CH = 3                                  # tiles per attention chunk (PSUM bank)
MAXI = 1024                             # max idxs per dma_gather call (HW limit)
P = 128


def _host_shard(edges):
    sub = np.asarray(edges[:, 4], dtype=np.int64)
    rel = np.asarray(edges[:, 2], dtype=np.int64)
    obj = np.asarray(edges[:, 5], dtype=np.int64)
    ridx = np.asarray(edges[:, 0], dtype=np.int64)
    half = (sub >= HALF).astype(np.int64)

    degA = np.bincount(obj[half == 0], minlength=N)
    degB = np.bincount(obj[half == 1], minlength=N)
    tot = degA + degB

    # ---- nodes -> cores: LPT greedy on total degree, cap OUT_ROWS ----
    order = np.argsort(-tot, kind="stable")
    core_load = np.zeros(NCORES, dtype=np.int64)
    core_cnt = np.zeros(NCORES, dtype=np.int64)
    ncore = np.zeros(N, dtype=np.int64)
    for n in order:
        k = int(np.argmin(np.where(core_cnt < OUT_ROWS, core_load, 1 << 60)))
        ncore[n] = k
        core_load[k] += tot[n]
        core_cnt[k] += 1

    # ---- per core: snake-deal nodes (by degree desc) into 49 windows ----
    nwin = np.zeros(N, dtype=np.int64)
    npos = np.zeros(N, dtype=np.int64)
    rowmap = np.full((NCORES, OUT_ROWS), -1, dtype=np.int64)
    for k in range(NCORES):
        mine = np.nonzero(ncore == k)[0]
        mine = mine[np.argsort(-tot[mine], kind="stable")]
        wfill = np.zeros(NWIN, dtype=np.int64)
        wi = 0
        direction = 1
        for n in mine:
            # snake over windows, skipping full ones
            tries = 0
            while wfill[wi] >= WIN:
                wi += direction
                if wi == NWIN or wi < 0:
                    direction = -direction
                    wi += direction
                tries += 1
                assert tries <= 2 * NWIN
            nwin[n] = wi
            npos[n] = wfill[wi]
            rowmap[k, wi * WIN + wfill[wi]] = n
            wfill[wi] += 1
            wi += direction
            if wi == NWIN or wi < 0:
                direction = -direction
                wi += direction

    # ---- per (core, window, half) edge counts -> uniform targets ----
    ek = ncore[obj]
    ew = nwin[obj]
    key = (ek * NWIN + ew) * 2 + half
    cnts = np.bincount(key, minlength=NCORES * NWIN * 2).reshape(
        NCORES, NWIN, 2)
    tgt = cnts.max(axis=0)              # [NWIN, 2]

    # ---- uniform slot layout ----
    # per group: [w0A w1A w2A padA | w0B w1B w2B padB], sections 128-aligned
    ngrp = (NWIN + G - 1) // G
    offs = np.zeros((NWIN, 2), dtype=np.int64)   # stretch start slot
    groups = []     # (c_start_tile, tA, tB, wlist)
    slot_win = []   # per-slot window id or -1 (uniform)
    cur = 0
    for g in range(ngrp):
        wlist = list(range(g * G, min((g + 1) * G, NWIN)))
        c_start = cur // P
        secs = []
        for h in (0, 1):
            sec0 = cur
            for w in wlist:
                offs[w, h] = cur
                cur += int(tgt[w, h])
            pad = (-(cur - sec0)) % P
            cur += pad
            secs.append((cur - sec0) // P)
        groups.append((c_start, secs[0], secs[1], wlist))
        wmap = np.full(cur - c_start * P, -1, dtype=np.int64)
        for h in (0, 1):
            for w in wlist:
                o = offs[w, h] - c_start * P
                wmap[o:o + tgt[w, h]] = w
        slot_win.append(wmap)
    ctot = cur // P
    slot_win = np.concatenate(slot_win)
    S = ctot * P // 16

    # ---- per-tile incidence lists (uniform across cores) ----
    tile_inc = []   # per tile: list of (w, col)
    inc_of = {}
    ncol = 0
    for ct in range(ctot):
        ws = [int(w) for w in
              np.unique(slot_win[ct * P:(ct + 1) * P]) if w >= 0]
        lst = []
        for w in ws:
            lst.append((w, ncol))
            inc_of.setdefault(w, []).append((ct, ncol))
            ncol += 1
        tile_inc.append(lst)
    first_inc = {w: v[0] for w, v in inc_of.items()}
    last_inc = {w: v[-1] for w, v in inc_of.items()}

    # ---- per-core slot data ----
    subs16 = np.zeros((NCORES, 16, S), dtype=np.int16)
    rels16 = np.zeros((NCORES, 16, S), dtype=np.int16)
    objc = np.full((NCORES, P, ncol), -1.0, dtype=np.float32)

    esel = npos[obj]
    erelc = rel * 64 + ridx
    eorder = np.lexsort((half, ew, ek))
    ksort, wsort, hsort = ek[eorder], ew[eorder], half[eorder]
    bkey = (ksort * NWIN + wsort) * 2 + hsort
    bounds = np.searchsorted(bkey, np.arange(NCORES * NWIN * 2 + 1))

    def wrap(dst, col0, vals):
        j = np.arange(len(vals))
        dst[j % 16, col0 + j // 16] = vals

    for k in range(NCORES):
        sub_slot = np.zeros(ctot * P, dtype=np.int64)
        rel_slot = np.zeros(ctot * P, dtype=np.int64)
        sel_slot = np.full(ctot * P, -1.0, dtype=np.float32)
        for w in range(NWIN):
            for h in (0, 1):
                bi = (k * NWIN + w) * 2 + h
                eidx = eorder[bounds[bi]:bounds[bi + 1]]
                n = len(eidx)
                o = offs[w, h]
                sub_slot[o:o + n] = sub[eidx] - h * HALF
                rel_slot[o:o + n] = erelc[eidx]
                sel_slot[o:o + n] = esel[eidx]
        # idx arrays: one 16-wrap run per gather section
        for (c_start, tA, tB, wlist) in groups:
            b0 = c_start * P
            wrap(subs16[k], b0 // 16, sub_slot[b0:b0 + tA * P])
            wrap(subs16[k], (b0 + tA * P) // 16,
                 sub_slot[b0 + tA * P:b0 + (tA + tB) * P])
            wrap(rels16[k], b0 // 16, rel_slot[b0:b0 + (tA + tB) * P])
        # incidence one-hot select columns
        for ct in range(ctot):
            sl = slice(ct * P, (ct + 1) * P)
            for (w, col) in tile_inc[ct]:
                objc[k, :, col] = np.where(slot_win[sl] == w,
                                           sel_slot[sl], -1.0)

    subs16 = np.tile(subs16, (1, 8, 1))
    rels16 = np.tile(rels16, (1, 8, 1))
    sched = dict(groups=groups, tile_inc=tile_inc, first_inc=first_inc,
                 last_inc=last_inc, ctot=ctot, ncol=ncol)
    return subs16, rels16, objc, rowmap, sched


def _build_graph(sched):
    import concourse.bass as bass
    import concourse.bacc as bacc
    import concourse.mybir as mybir
    from concourse.tile import TileContext

    f32 = mybir.dt.float32
    bf16 = mybir.dt.bfloat16
    i16 = mybir.dt.int16
    AF = mybir.ActivationFunctionType
    Alu = mybir.AluOpType

    ctot = sched["ctot"]
    ncol = sched["ncol"]
    groups = sched["groups"]
    tile_inc = sched["tile_inc"]
    first_inc = sched["first_inc"]
    last_inc = sched["last_inc"]
    S = ctot * P // 16

    nc = bacc.Bacc(dynamic_dma_scratch_size=65536)
    hid_rm = nc.declare_dram_parameter("hid_rm", [ROWS_T, D], bf16, isOutput=False)
    crel_d = nc.declare_dram_parameter("crel", [CREL_ROWS, 2 * D], bf16,
                                       isOutput=False)
    ws_p = nc.declare_dram_parameter("ws_p", [D, D], bf16, isOutput=False)
    wh_p = nc.declare_dram_parameter("wh_p", [D, D], bf16, isOutput=False)
    wa_p = nc.declare_dram_parameter("wa_p", [D, 1], bf16, isOutput=False)
    sub_i = nc.declare_dram_parameter("sub_i", [P, S], i16, isOutput=False)
    rel_i = nc.declare_dram_parameter("rel_i", [P, S], i16, isOutput=False)
    obj_f = nc.declare_dram_parameter("obj_f", [P, ncol], f32, isOutput=False)
    out_ext = nc.declare_dram_parameter("out", [OUT_ROWS, D], bf16, isOutput=True)

    with TileContext(nc) as tc:
        with (
            tc.tile_pool(name="const", bufs=1) as cpool,
            tc.tile_pool(name="work", bufs=2) as wpool,
            tc.tile_pool(name="psum", bufs=2, space="PSUM") as ppool,
        ):
            # ---- constants ----
            chan_i = cpool.tile([P, 1], mybir.dt.int32)
            nc.gpsimd.iota(chan_i[:], pattern=[[0, 1]], base=0, channel_multiplier=1)
            chan_f = cpool.tile([P, 1], f32)
            nc.vector.tensor_copy(chan_f[:], chan_i[:])

            iota_i = cpool.tile([P, P], i16)
            nc.gpsimd.iota(iota_i[:], pattern=[[1, P]], base=0, channel_multiplier=0)
            iota_b = cpool.tile([P, P], bf16)
            nc.vector.tensor_copy(iota_b[:], iota_i[:])

            i128 = cpool.tile([P, P], bf16)
            nc.vector.tensor_scalar(out=i128[:], in0=iota_b[:],
                                    scalar1=chan_f[:], scalar2=None,
                                    op0=Alu.is_equal)

            # ---- weights ----
            ws_sb = cpool.tile([P, D], bf16)
            nc.sync.dma_start(out=ws_sb[:], in_=ws_p[:])
            wh_sb = cpool.tile([P, D], bf16)
            nc.sync.dma_start(out=wh_sb[:], in_=wh_p[:])
            wa_sb = cpool.tile([P, 1], bf16)
            nc.sync.dma_start(out=wa_sb[:], in_=wa_p[:])

            # ---- edge index arrays ----
            sub_s = cpool.tile([P, S], i16)
            nc.sync.dma_start(out=sub_s[:], in_=sub_i[:])
            rel_s = cpool.tile([P, S], i16)
            nc.sync.dma_start(out=rel_s[:], in_=rel_i[:])
            obj_s = cpool.tile([P, ncol], f32)
            nc.sync.dma_start(out=obj_s[:], in_=obj_f[:])

            def chunked_gather(dst_tile, src_ap, idxs_tile, idx_col0, t_off, n,
                               esize):
                done = 0
                while done < n:
                    cn = min(MAXI, n - done)
                    ct0 = t_off + done // P
                    nc.gpsimd.dma_gather(
                        out_ap=dst_tile[:, ct0:ct0 + cn // P, :],
                        in_ap=src_ap,
                        idxs_ap=idxs_tile[:, idx_col0 + done // 16:
                                          idx_col0 + (done + cn) // 16],
                        num_idxs=cn, num_idxs_reg=cn, elem_size=esize)
                    done += cn

            # ---- edge processing ----
            evict_n = 0
            agg = {}
            for g_idx, (c_start, tA, tB, wlist) in enumerate(groups):
                g0w = wlist[0]
                nw = len(wlist)
                T = tA + tB
                nA = tA * P
                nB = tB * P
                s0 = c_start * P // 16

                g_t = wpool.tile([P, T, D], bf16, tag="g_hs", bufs=3)
                r_t = wpool.tile([P, T, 2 * D], bf16, tag="g_cr", bufs=3)
                chunked_gather(r_t, crel_d[:], rel_s, s0, 0, nA + nB, 2 * D)
                if tA:
                    chunked_gather(g_t, hid_rm[0:HALF, :], sub_s, s0, 0, nA, D)
                if tB:
                    chunked_gather(g_t, hid_rm[HALF:ROWS_T, :], sub_s,
                                   s0 + nA // 16, tA, nB, D)

                # ---- attention ----
                alpha = wpool.tile([P, T], f32, tag="alpha", bufs=3)
                lgt = ppool.tile([P, P], f32, tag="lgt", bufs=1)
                for j0 in range(0, T, CH):
                    nb = min(CH, T - j0)
                    psT = ppool.tile([P, CH, D], bf16, tag="psT", bufs=2)
                    for j in range(nb):
                        nc.tensor.matmul(psT[:, j, :], lhsT=g_t[:, j0 + j, :],
                                         rhs=i128[:], is_transpose=True,
                                         start=True, stop=True)
                    hsT_sb = wpool.tile([P, CH, D], bf16, tag="hsT", bufs=3)
                    nc.vector.tensor_copy(hsT_sb[:, 0:nb, :], psT[:, 0:nb, :])
                    preL = ppool.tile([P, CH * D + CH], f32, tag="preL", bufs=2)
                    for j in range(nb):
                        nc.tensor.matmul(preL[:, j * D:(j + 1) * D],
                                         lhsT=ws_sb[:], rhs=hsT_sb[:, j, :],
                                         start=True, stop=False)
                        nc.tensor.matmul(preL[:, j * D:(j + 1) * D],
                                         lhsT=r_t[:, j0 + j, 0:D],
                                         rhs=i128[:], start=False, stop=True)
                    s4 = wpool.tile([P, CH, D], bf16, tag="s4", bufs=3)
                    nc.scalar.activation(s4[:, 0:nb, :], preL[:, 0:nb * D],
                                         AF.Relu)
                    for j in range(nb):
                        nc.tensor.matmul(
                            lgt[:, j0 + j:j0 + j + 1],
                            lhsT=s4[:, j, :], rhs=wa_sb[:],
                            start=True, stop=True)
                    nc.scalar.activation(alpha[:, j0:j0 + nb],
                                         lgt[:, j0:j0 + nb],
                                         AF.Sigmoid)

                # ---- aggregation over (tile, window) incidences ----
                for c in range(T):
                    ct = c_start + c
                    for (w, col) in tile_inc[ct]:
                        oh = wpool.tile([P, P], bf16, tag="oh", bufs=8)
                        nc.vector.tensor_scalar(
                            out=oh[:], in0=iota_b[:],
                            scalar1=obj_s[:, col:col + 1],
                            scalar2=alpha[:, c:c + 1],
                            op0=Alu.is_equal, op1=Alu.mult)
                        if (ct, col) == first_inc[w]:
                            agg[w] = ppool.tile([P, D], f32, tag="agg",
                                                bufs=3, name=f"agg_{w}")
                        nc.tensor.matmul(agg[w][:], lhsT=g_t[:, c, :],
                                         rhs=oh[:],
                                         start=((ct, col) == first_inc[w]),
                                         stop=False)
                        nc.tensor.matmul(agg[w][:], lhsT=r_t[:, c, D:2 * D],
                                         rhs=oh[:], start=False,
                                         stop=((ct, col) == last_inc[w]))

                # ---- epilogue: out = relu(aggT.T @ Wh) ----
                agg_sb = wpool.tile([P, G, D], bf16, tag="aggsb", bufs=3)
                for wi, w in enumerate(wlist):
                    if evict_n % 2 == 0:
                        nc.vector.tensor_copy(agg_sb[:, wi, :], agg[w][:])
                    else:
                        nc.scalar.copy(agg_sb[:, wi, :], agg[w][:])
                    evict_n += 1
                    del agg[w]
                outp = ppool.tile([P, CH * D + CH], f32, tag="preL", bufs=2)
                for wi in range(nw):
                    nc.tensor.matmul(outp[:, wi * D:(wi + 1) * D],
                                     lhsT=agg_sb[:, wi, :],
                                     rhs=wh_sb[:], start=True, stop=True)
                ostage = wpool.tile([P, G, D], bf16, tag="ostage", bufs=3)
                nc.scalar.activation(ostage[:, 0:nw, :], outp[:, 0:nw * D],
                                     AF.Relu)
                out_ap = bass.AP(out_ext[:].tensor, g0w * P * D,
                                 [[D, P], [P * D, nw], [1, D]])
                nc.sync.dma_start(out=out_ap, in_=ostage[:, 0:nw, :])

    nc.compile()
    return nc


def prepare(q_rel, hidden, edges, rela_embed, Ws, Wr, Wqr_w, Wqr_b, Wa, Wh,
            n_node=None):
    """Build the Bass graph and the 8 per-core input maps."""
    import ml_dtypes

    bf = ml_dtypes.bfloat16
    q_rel = np.asarray(q_rel, dtype=np.int64)
    hidden = np.asarray(hidden, dtype=np.float32)
    edges = np.asarray(edges)
    rela = np.asarray(rela_embed, dtype=np.float32)

    subs16, rels16, objc, rowmap, sched = _host_shard(edges)
    nc = _build_graph(sched)

    hid_rm = np.zeros((ROWS_T, D), dtype=bf)
    hid_rm[:N] = hidden.astype(bf)

    attn_rel = rela @ np.asarray(Wr, dtype=np.float32)
    attn_q = (rela[q_rel] @ np.asarray(Wqr_w, dtype=np.float32)
              + np.asarray(Wqr_b, dtype=np.float32))
    crel = np.zeros((CREL_ROWS, 2 * D), dtype=np.float32)
    nrel = rela.shape[0]
    crel_attn = (attn_rel[:, None, :] + attn_q[None, :, :])
    crel[:nrel * 64, 0:D] = crel_attn.reshape(nrel * 64, D)
    crel[:nrel * 64, D:2 * D] = np.repeat(rela, 64, axis=0)
    crel = np.ascontiguousarray(crel.astype(bf))

    ws_b = np.ascontiguousarray(np.asarray(Ws, dtype=np.float32).astype(bf))
    wh_b = np.ascontiguousarray(np.asarray(Wh, dtype=np.float32).astype(bf))
    wa_b = np.ascontiguousarray(
        np.asarray(Wa, dtype=np.float32).reshape(D, 1).astype(bf))

    in_maps = []
    for k in range(NCORES):
        in_maps.append({
            "hid_rm": hid_rm,
            "crel": crel,
            "ws_p": ws_b,
            "wh_p": wh_b,
            "wa_p": wa_b,
            "sub_i": subs16[k],
            "rel_i": rels16[k],
            "obj_f": objc[k],
        })
    return nc, in_maps, rowmap


def assemble(results, rowmap):
    out = np.zeros((N, D), dtype=np.float32)
    for k in range(NCORES):
        rows = np.asarray(results[k]["out"], dtype=np.float32)
        valid = rowmap[k] >= 0
        out[rowmap[k][valid]] = rows[valid]
    return out


def kernel(q_rel, hidden, edges, rela_embed, Ws, Wr, Wqr_w, Wqr_b, Wa, Wh, n_node):
    from concourse.bass_utils import run_bass_kernel_spmd

    nc, in_maps, rowmap = prepare(q_rel, hidden, edges, rela_embed, Ws, Wr,
                                  Wqr_w, Wqr_b, Wa, Wh, n_node)
    res = run_bass_kernel_spmd(nc, in_maps, list(range(NCORES)))
    return assemble(res.results, rowmap)


if __name__ == "__main__":
    import reference

    inputs = reference.setup_inputs()
    inputs = {k: np.asarray(v) for k, v in inputs.items()}
    got = kernel(**inputs)
    exp = np.asarray(reference.reference(**inputs))
    err = np.abs(got - exp).max() / (np.abs(exp).max() + 1e-9)
    print("rel err:", err)


# revision 8
# speedup vs baseline: 2.1135x; 1.0029x over previous
"""AdaProp GNN message-passing kernel for 8 TRN2 NeuronCores.

v4 = v3 graph + balanced host scheduling:
- nodes are assigned to cores/windows by degree-balancing (LPT + 2-D greedy),
  with the output unpermuted on the host, so per-(window,half) slot targets
  are uniform across cores at 1-slot granularity (no 128-padding per window);
- only gather sections (the [A-half | B-half] runs of each G-window group)
  are 128-aligned; tiles straddle windows, handled by per-tile incidence
  lists (one one-hot + matmul pair per (tile, window) incidence).

Per edge, two dma_gather rows: hs = hidden_rm[sub] (256B, A/B halves) and
crel[rel*64+r_idx] = [rela@Wr + rela[q_rel]@Wqr + b | rela_raw] (512B,
host-built). Attention runs in transposed [a, slot] space (PE identity-
transpose + Ws matmul + identity-add of crel_attn), relu on Activation,
logit = relu(preT).T @ Wa on PE (slot-partitioned), sigmoid per chunk.
Message aggregate aggT[d, node] += msg.T @ oh in PSUM; out = relu(aggT.T@Wh).
"""

import numpy as np

N, E, B, D = 50000, 500_000, 64, 128
NCORES = 8
WIN = 128
NWIN = 49                               # windows per core
OUT_ROWS = NWIN * WIN                   # 6272 output rows per core
NPC = OUT_ROWS                          # rows fetched per core (permuted)
ROWS_T = 50176                          # hidden_rm rows (50000 padded)
HALF = ROWS_T // 2                      # 25088 (< 32768 so int16 idx works)
CREL_T = 201
CREL_ROWS = CREL_T * 128
G = 3                                   # windows per group
CH = 3                                  # tiles per attention chunk (PSUM bank)
MAXI = 1024                             # max idxs per dma_gather call (HW limit)
P = 128


def _host_shard(edges):
    sub = np.asarray(edges[:, 4], dtype=np.int64)
    rel = np.asarray(edges[:, 2], dtype=np.int64)
    obj = np.asarray(edges[:, 5], dtype=np.int64)
    ridx = np.asarray(edges[:, 0], dtype=np.int64)
    half = (sub >= HALF).astype(np.int64)

    degA = np.bincount(obj[half == 0], minlength=N)
    degB = np.bincount(obj[half == 1], minlength=N)
    tot = degA + degB

    # ---- nodes -> cores: LPT greedy on total degree, cap OUT_ROWS ----
    order = np.argsort(-tot, kind="stable")
    core_load = np.zeros(NCORES, dtype=np.int64)
    core_cnt = np.zeros(NCORES, dtype=np.int64)
    ncore = np.zeros(N, dtype=np.int64)
    for n in order:
        k = int(np.argmin(np.where(core_cnt < OUT_ROWS, core_load, 1 << 60)))
        ncore[n] = k
        core_load[k] += tot[n]
        core_cnt[k] += 1

    # ---- per core: greedy 2-D balance of (A, B) edge sums per window ----
    nwin = np.zeros(N, dtype=np.int64)
    npos = np.zeros(N, dtype=np.int64)
    rowmap = np.full((NCORES, OUT_ROWS), -1, dtype=np.int64)
    for k in range(NCORES):
        mine = np.nonzero(ncore == k)[0]
        mine = mine[np.argsort(-tot[mine], kind="stable")]
        mA = max(1.0, degA[mine].sum() / NWIN)
        mB = max(1.0, degB[mine].sum() / NWIN)
        wA = np.zeros(NWIN)
        wB = np.zeros(NWIN)
        wfill = np.zeros(NWIN, dtype=np.int64)
        for n in mine:
            score = np.maximum((wA + degA[n]) / mA, (wB + degB[n]) / mB)
            score[wfill >= WIN] = 1e18
            wi = int(np.argmin(score))
            nwin[n] = wi
            npos[n] = wfill[wi]
            rowmap[k, wi * WIN + wfill[wi]] = n
            wA[wi] += degA[n]
            wB[wi] += degB[n]
            wfill[wi] += 1

    # ---- per (core, window, half) edge counts -> uniform targets ----
    ek = ncore[obj]
    ew = nwin[obj]
    key = (ek * NWIN + ew) * 2 + half
    cnts = np.bincount(key, minlength=NCORES * NWIN * 2).reshape(
        NCORES, NWIN, 2)
    tgt = cnts.max(axis=0)              # [NWIN, 2]

    # ---- uniform slot layout ----
    # per group: [w0A w1A w2A padA | w0B w1B w2B padB], sections 128-aligned
    ngrp = (NWIN + G - 1) // G
    offs = np.zeros((NWIN, 2), dtype=np.int64)   # stretch start slot
    groups = []     # (c_start_tile, tA, tB, wlist)
    slot_win = []   # per-slot window id or -1 (uniform)
    cur = 0
    for g in range(ngrp):
        wlist = list(range(g * G, min((g + 1) * G, NWIN)))
        c_start = cur // P
        secs = []
        for h in (0, 1):
            sec0 = cur
            for w in wlist:
                offs[w, h] = cur
                cur += int(tgt[w, h])
            pad = (-(cur - sec0)) % P
            cur += pad
            secs.append((cur - sec0) // P)
        groups.append((c_start, secs[0], secs[1], wlist))
        wmap = np.full(cur - c_start * P, -1, dtype=np.int64)
        for h in (0, 1):
            for w in wlist:
                o = offs[w, h] - c_start * P
                wmap[o:o + tgt[w, h]] = w
        slot_win.append(wmap)
    ctot = cur // P
    slot_win = np.concatenate(slot_win)
    S = ctot * P // 16

    # ---- per-tile incidence lists (uniform across cores) ----
    tile_inc = []   # per tile: list of (w, col)
    inc_of = {}
    ncol = 0
    for ct in range(ctot):
        ws = [int(w) for w in
              np.unique(slot_win[ct * P:(ct + 1) * P]) if w >= 0]
        lst = []
        for w in ws:
            lst.append((w, ncol))
            inc_of.setdefault(w, []).append((ct, ncol))
            ncol += 1
        tile_inc.append(lst)
    first_inc = {w: v[0] for w, v in inc_of.items()}
    last_inc = {w: v[-1] for w, v in inc_of.items()}

    # ---- per-core slot data ----
    subs16 = np.zeros((NCORES, 16, S), dtype=np.int16)
    rels16 = np.zeros((NCORES, 16, S), dtype=np.int16)
    objc = np.full((NCORES, P, ncol), -1.0, dtype=np.float32)

    esel = npos[obj]
    erelc = rel * 64 + ridx
    eorder = np.lexsort((half, ew, ek))
    ksort, wsort, hsort = ek[eorder], ew[eorder], half[eorder]
    bkey = (ksort * NWIN + wsort) * 2 + hsort
    bounds = np.searchsorted(bkey, np.arange(NCORES * NWIN * 2 + 1))

    def wrap(dst, col0, vals):
        j = np.arange(len(vals))
        dst[j % 16, col0 + j // 16] = vals

    for k in range(NCORES):
        sub_slot = np.zeros(ctot * P, dtype=np.int64)
        rel_slot = np.zeros(ctot * P, dtype=np.int64)
        sel_slot = np.full(ctot * P, -1.0, dtype=np.float32)
        for w in range(NWIN):
            for h in (0, 1):
                bi = (k * NWIN + w) * 2 + h
                eidx = eorder[bounds[bi]:bounds[bi + 1]]
                n = len(eidx)
                o = offs[w, h]
                sub_slot[o:o + n] = sub[eidx] - h * HALF
                rel_slot[o:o + n] = erelc[eidx]
                sel_slot[o:o + n] = esel[eidx]
        # idx arrays: one 16-wrap run per gather section
        for (c_start, tA, tB, wlist) in groups:
            b0 = c_start * P
            wrap(subs16[k], b0 // 16, sub_slot[b0:b0 + tA * P])
            wrap(subs16[k], (b0 + tA * P) // 16,
                 sub_slot[b0 + tA * P:b0 + (tA + tB) * P])
            wrap(rels16[k], b0 // 16, rel_slot[b0:b0 + (tA + tB) * P])
        # incidence one-hot select columns
        for ct in range(ctot):
            sl = slice(ct * P, (ct + 1) * P)
            for (w, col) in tile_inc[ct]:
                objc[k, :, col] = np.where(slot_win[sl] == w,
                                           sel_slot[sl], -1.0)

    subs16 = np.tile(subs16, (1, 8, 1))
    rels16 = np.tile(rels16, (1, 8, 1))
    sched = dict(groups=groups, tile_inc=tile_inc, first_inc=first_inc,
                 last_inc=last_inc, ctot=ctot, ncol=ncol)
    return subs16, rels16, objc, rowmap, sched


def _build_graph(sched):
    import concourse.bass as bass
    import concourse.bacc as bacc
    import concourse.mybir as mybir
    from concourse.tile import TileContext

    f32 = mybir.dt.float32
    bf16 = mybir.dt.bfloat16
    i16 = mybir.dt.int16
    AF = mybir.ActivationFunctionType
    Alu = mybir.AluOpType

    ctot = sched["ctot"]
    ncol = sched["ncol"]
    groups = sched["groups"]
    tile_inc = sched["tile_inc"]
    first_inc = sched["first_inc"]
    last_inc = sched["last_inc"]
    S = ctot * P // 16

    nc = bacc.Bacc(dynamic_dma_scratch_size=65536)
    hid_rm = nc.declare_dram_parameter("hid_rm", [ROWS_T, D], bf16, isOutput=False)
    crel_d = nc.declare_dram_parameter("crel", [CREL_ROWS, 2 * D], bf16,
                                       isOutput=False)
    ws_p = nc.declare_dram_parameter("ws_p", [D, D], bf16, isOutput=False)
    wh_p = nc.declare_dram_parameter("wh_p", [D, D], bf16, isOutput=False)
    wa_p = nc.declare_dram_parameter("wa_p", [D, 1], bf16, isOutput=False)
    sub_i = nc.declare_dram_parameter("sub_i", [P, S], i16, isOutput=False)
    rel_i = nc.declare_dram_parameter("rel_i", [P, S], i16, isOutput=False)
    obj_f = nc.declare_dram_parameter("obj_f", [P, ncol], f32, isOutput=False)
    out_ext = nc.declare_dram_parameter("out", [OUT_ROWS, D], bf16, isOutput=True)

    with TileContext(nc) as tc:
        with (
            tc.tile_pool(name="const", bufs=1) as cpool,
            tc.tile_pool(name="work", bufs=2) as wpool,
            tc.tile_pool(name="psum", bufs=2, space="PSUM") as ppool,
        ):
            # ---- constants ----
            chan_i = cpool.tile([P, 1], mybir.dt.int32)
            nc.gpsimd.iota(chan_i[:], pattern=[[0, 1]], base=0, channel_multiplier=1)
            chan_f = cpool.tile([P, 1], f32)
            nc.vector.tensor_copy(chan_f[:], chan_i[:])

            iota_i = cpool.tile([P, P], i16)
            nc.gpsimd.iota(iota_i[:], pattern=[[1, P]], base=0, channel_multiplier=0)
            iota_b = cpool.tile([P, P], bf16)
            nc.vector.tensor_copy(iota_b[:], iota_i[:])

            i128 = cpool.tile([P, P], bf16)
            nc.vector.tensor_scalar(out=i128[:], in0=iota_b[:],
                                    scalar1=chan_f[:], scalar2=None,
                                    op0=Alu.is_equal)

            # ---- weights ----
            ws_sb = cpool.tile([P, D], bf16)
            nc.sync.dma_start(out=ws_sb[:], in_=ws_p[:])
            wh_sb = cpool.tile([P, D], bf16)
            nc.sync.dma_start(out=wh_sb[:], in_=wh_p[:])
            wa_sb = cpool.tile([P, 1], bf16)
            nc.sync.dma_start(out=wa_sb[:], in_=wa_p[:])

            # ---- edge index arrays ----
            sub_s = cpool.tile([P, S], i16)
            nc.sync.dma_start(out=sub_s[:], in_=sub_i[:])
            rel_s = cpool.tile([P, S], i16)
            nc.sync.dma_start(out=rel_s[:], in_=rel_i[:])
            obj_s = cpool.tile([P, ncol], f32)
            nc.sync.dma_start(out=obj_s[:], in_=obj_f[:])

            def chunked_gather(dst_tile, src_ap, idxs_tile, idx_col0, t_off, n,
                               esize):
                done = 0
                while done < n:
                    cn = min(MAXI, n - done)
                    ct0 = t_off + done // P
                    nc.gpsimd.dma_gather(
                        out_ap=dst_tile[:, ct0:ct0 + cn // P, :],
                        in_ap=src_ap,
                        idxs_ap=idxs_tile[:, idx_col0 + done // 16:
                                          idx_col0 + (done + cn) // 16],
                        num_idxs=cn, num_idxs_reg=cn, elem_size=esize)
                    done += cn

            # ---- edge processing ----
            evict_n = 0
            agg = {}
            for g_idx, (c_start, tA, tB, wlist) in enumerate(groups):
                g0w = wlist[0]
                nw = len(wlist)
                T = tA + tB
                nA = tA * P
                nB = tB * P
                s0 = c_start * P // 16

                g_t = wpool.tile([P, T, D], bf16, tag="g_hs", bufs=3)
                r_t = wpool.tile([P, T, 2 * D], bf16, tag="g_cr", bufs=3)
                chunked_gather(r_t, crel_d[:], rel_s, s0, 0, nA + nB, 2 * D)
                if tA:
                    chunked_gather(g_t, hid_rm[0:HALF, :], sub_s, s0, 0, nA, D)
                if tB:
                    chunked_gather(g_t, hid_rm[HALF:ROWS_T, :], sub_s,
                                   s0 + nA // 16, tA, nB, D)

                # ---- attention ----
                alpha = wpool.tile([P, T], f32, tag="alpha", bufs=3)
                lgt = ppool.tile([P, P], f32, tag="lgt", bufs=1)
                for j0 in range(0, T, CH):
                    nb = min(CH, T - j0)
                    psT = ppool.tile([P, CH, D], bf16, tag="psT", bufs=2)
                    for j in range(nb):
                        nc.tensor.matmul(psT[:, j, :], lhsT=g_t[:, j0 + j, :],
                                         rhs=i128[:], is_transpose=True,
                                         start=True, stop=True)
                    hsT_sb = wpool.tile([P, CH, D], bf16, tag="hsT", bufs=3)
                    nc.vector.tensor_copy(hsT_sb[:, 0:nb, :], psT[:, 0:nb, :])
                    preL = ppool.tile([P, CH * D + CH], f32, tag="preL", bufs=2)
                    for j in range(nb):
                        nc.tensor.matmul(preL[:, j * D:(j + 1) * D],
                                         lhsT=ws_sb[:], rhs=hsT_sb[:, j, :],
                                         start=True, stop=False)
                        nc.tensor.matmul(preL[:, j * D:(j + 1) * D],
                                         lhsT=r_t[:, j0 + j, 0:D],
                                         rhs=i128[:], start=False, stop=True)
                    s4 = wpool.tile([P, CH, D], bf16, tag="s4", bufs=3)
                    nc.scalar.activation(s4[:, 0:nb, :], preL[:, 0:nb * D],
                                         AF.Relu)
                    for j in range(nb):
                        nc.tensor.matmul(
                            lgt[:, j0 + j:j0 + j + 1],
                            lhsT=s4[:, j, :], rhs=wa_sb[:],
                            start=True, stop=True)
                    nc.scalar.activation(alpha[:, j0:j0 + nb],
                                         lgt[:, j0:j0 + nb],
                                         AF.Sigmoid)

                # ---- aggregation over (tile, window) incidences ----
                for c in range(T):
                    ct = c_start + c
                    for (w, col) in tile_inc[ct]:
                        oh = wpool.tile([P, P], bf16, tag="oh", bufs=8)
                        nc.vector.tensor_scalar(
                            out=oh[:], in0=iota_b[:],
                            scalar1=obj_s[:, col:col + 1],
                            scalar2=alpha[:, c:c + 1],
                            op0=Alu.is_equal, op1=Alu.mult)
                        if (ct, col) == first_inc[w]:
                            agg[w] = ppool.tile([P, D], f32, tag="agg",
                                                bufs=3, name=f"agg_{w}")
                        nc.tensor.matmul(agg[w][:], lhsT=g_t[:, c, :],
                                         rhs=oh[:],
                                         start=((ct, col) == first_inc[w]),
                                         stop=False)
                        nc.tensor.matmul(agg[w][:], lhsT=r_t[:, c, D:2 * D],
                                         rhs=oh[:], start=False,
                                         stop=((ct, col) == last_inc[w]))

                # ---- epilogue: out = relu(aggT.T @ Wh) ----
                agg_sb = wpool.tile([P, G, D], bf16, tag="aggsb", bufs=3)
                for wi, w in enumerate(wlist):
                    if evict_n % 2 == 0:
                        nc.vector.tensor_copy(agg_sb[:, wi, :], agg[w][:])
                    else:
                        nc.scalar.copy(agg_sb[:, wi, :], agg[w][:])
                    evict_n += 1
                    del agg[w]
                outp = ppool.tile([P, CH * D + CH], f32, tag="preL", bufs=2)
                for wi in range(nw):
                    nc.tensor.matmul(outp[:, wi * D:(wi + 1) * D],
                                     lhsT=agg_sb[:, wi, :],
                                     rhs=wh_sb[:], start=True, stop=True)
                ostage = wpool.tile([P, G, D], bf16, tag="ostage", bufs=3)
                nc.scalar.activation(ostage[:, 0:nw, :], outp[:, 0:nw * D],
                                     AF.Relu)
                out_ap = bass.AP(out_ext[:].tensor, g0w * P * D,
                                 [[D, P], [P * D, nw], [1, D]])
                nc.sync.dma_start(out=out_ap, in_=ostage[:, 0:nw, :])

    nc.compile()
    return nc


def prepare(q_rel, hidden, edges, rela_embed, Ws, Wr, Wqr_w, Wqr_b, Wa, Wh,
            n_node=None):
    """Build the Bass graph and the 8 per-core input maps."""
    import ml_dtypes

    bf = ml_dtypes.bfloat16
    q_rel = np.asarray(q_rel, dtype=np.int64)
    hidden = np.asarray(hidden, dtype=np.float32)
    edges = np.asarray(edges)
    rela = np.asarray(rela_embed, dtype=np.float32)

    subs16, rels16, objc, rowmap, sched = _host_shard(edges)
    nc = _build_graph(sched)

    hid_rm = np.zeros((ROWS_T, D), dtype=bf)
    hid_rm[:N] = hidden.astype(bf)

    attn_rel = rela @ np.asarray(Wr, dtype=np.float32)
    attn_q = (rela[q_rel] @ np.asarray(Wqr_w, dtype=np.float32)
              + np.asarray(Wqr_b, dtype=np.float32))
    crel = np.zeros((CREL_ROWS, 2 * D), dtype=np.float32)
    nrel = rela.shape[0]
    crel_attn = (attn_rel[:, None, :] + attn_q[None, :, :])
    crel[:nrel * 64, 0:D] = crel_attn.reshape(nrel * 64, D)
    crel[:nrel * 64, D:2 * D] = np.repeat(rela, 64, axis=0)
    crel = np.ascontiguousarray(crel.astype(bf))

    ws_b = np.ascontiguousarray(np.asarray(Ws, dtype=np.float32).astype(bf))
    wh_b = np.ascontiguousarray(np.asarray(Wh, dtype=np.float32).astype(bf))
    wa_b = np.ascontiguousarray(
        np.asarray(Wa, dtype=np.float32).reshape(D, 1).astype(bf))

    in_maps = []
    for k in range(NCORES):
        in_maps.append({
            "hid_rm": hid_rm,
            "crel": crel,
            "ws_p": ws_b,
            "wh_p": wh_b,
            "wa_p": wa_b,
            "sub_i": subs16[k],
            "rel_i": rels16[k],
            "obj_f": objc[k],
        })
    return nc, in_maps, rowmap


def assemble(results, rowmap):
    out = np.zeros((N, D), dtype=np.float32)
    for k in range(NCORES):
        rows = np.asarray(results[k]["out"], dtype=np.float32)
        valid = rowmap[k] >= 0
        out[rowmap[k][valid]] = rows[valid]
    return out


def kernel(q_rel, hidden, edges, rela_embed, Ws, Wr, Wqr_w, Wqr_b, Wa, Wh, n_node):
    from concourse.bass_utils import run_bass_kernel_spmd

    nc, in_maps, rowmap = prepare(q_rel, hidden, edges, rela_embed, Ws, Wr,
                                  Wqr_w, Wqr_b, Wa, Wh, n_node)
    res = run_bass_kernel_spmd(nc, in_maps, list(range(NCORES)))
    return assemble(res.results, rowmap)


if __name__ == "__main__":
    import reference

    inputs = reference.setup_inputs()
    inputs = {k: np.asarray(v) for k, v in inputs.items()}
    got = kernel(**inputs)
    exp = np.asarray(reference.reference(**inputs))
    err = np.abs(got - exp).max() / (np.abs(exp).max() + 1e-9)
    print("rel err:", err)
